# revision 1
# baseline (speedup 1.0000x reference)
"""Depthwise 3x3 + pointwise 1x1 conv (both 4-bit fake-quant weights) on 8 trn2 cores.

Strategy: data-parallel over batch (32 -> 8 cores x 4 images).
Host pre-pads x to 58x58 (zero border) and converts it to fp16, so every DMA
load is one fully contiguous ~6.7KB-per-partition transfer and no on-device
dtype conversion of the input is needed.
Per core: channels-on-partitions layout (384 ch = 3 groups of 128).
 - depthwise: 9 taps, split by output rows between TensorE and VectorE
   (per-image/per-group row split in HS_PE, tuned for engine balance):
   * TensorE rows [0, HS): one diagonal-weight fp16 matmul per tap,
     accumulated in a PSUM bank over the 9 taps (8-row/448-col chunks),
     evacuated to fp16 on ScalarE.
   * VectorE rows [HS, 56): per-tap tensor_scalar products (4x DVE mode,
     per-partition tap scalars) + tensor_tensor adds (2x mode). ScalarE
     makes a shifted-by-one fp16 copy of the band so the center-column
     (dw=1) taps also read 4-byte-aligned.
 - pointwise: dense fp16 matmuls on TensorE (K=384 = 3x128 accumulated in
   fp32 PSUM), bias added on ScalarE (Identity activation + per-partition
   bias AP) during PSUM evacuation, stored as fp32.
Weight fake-quant is replicated on host in fp32 numpy (exact match to the
reference semantics: forward value of the STE round).
"""

import numpy as np

# Problem shape (hardcoded per contract).
B_TOTAL, C, H, W = 32, 384, 56, 56
N_CORES = 8
B = B_TOTAL // N_CORES          # images per core
HP = H + 2                      # padded spatial
NPAD = HP * HP
CG = C // 128                   # channel groups (contraction groups)
MG = C // 128                   # output-channel groups
P = 128

# Depthwise rows computed on TensorE, per (image, channel-group); the rest of
# the 56 rows go to VectorE. Image 0 leans on VectorE (PE starts before DVE
# has data), image 3 leans on TensorE (so the kernel tail isn't gated on the
# last VectorE band).
HS_PE = [
    [32, 32, 24],
    [32, 32, 24],
    [32, 32, 24],
    [40, 40, 32],
]
BUFS = {"xpad": 6, "x16b": 4, "tmp": 4, "ype": 9, "ydv": 9, "zst": 6,
        "dwps": 4, "pwps": 4}
CHUNK_ROWS = 8                  # rows per PSUM chunk (fp32 bank: 512 elems)
CHUNK = CHUNK_ROWS * W          # 448 columns


def _row_chunks(total, step=CHUNK_ROWS):
    out, r = [], 0
    while r < total:
        n = min(step, total - r)
        out.append((r, n))
        r += n
    return out

TAPS = [(dh, dw) for dh in range(3) for dw in range(3)]

WEIGHT_BITS = 4
SCALE_MIN = np.float32(2e-16)


def _fake_quant(w: np.ndarray, bits: int = WEIGHT_BITS) -> np.ndarray:
    """Forward value of brevitas-style per-channel symmetric narrow int quant."""
    w = w.astype(np.float32)
    qmax = np.float32(2 ** (bits - 1) - 1)
    absmax = np.max(np.abs(w.reshape(w.shape[0], -1)), axis=1)
    scale = np.maximum(absmax / qmax, SCALE_MIN).astype(np.float32)
    scale = scale.reshape((-1,) + (1,) * (w.ndim - 1))
    q = np.clip(np.round(w / scale), -qmax, qmax).astype(np.float32) * scale
    return q.astype(np.float32)


def _build_nc(reps: int = 1, hw_loop: int = 0):
    import concourse.bass as bass  # noqa: F401
    import concourse.tile as tile
    from concourse import bacc, mybir

    dt = mybir.dt
    f32, f16, f32r = dt.float32, dt.float16, dt.float32r
    Alu = mybir.AluOpType
    Act = mybir.ActivationFunctionType

    nc = bacc.Bacc("TRN2", target_bir_lowering=False, debug=False,
                   num_devices=N_CORES)

    # x arrives host-padded and host-converted: [B, C, 58, 58] fp16,
    # zero borders.
    x_d = nc.dram_tensor("x", [B, C, HP, HP], f16, kind="ExternalInput").ap()
    dwdiag_d = nc.dram_tensor("dwdiag", [P, CG * 9 * P], f16,
                              kind="ExternalInput").ap()
    pwT_d = nc.dram_tensor("pwT", [P, CG * MG * P], f16,
                           kind="ExternalInput").ap()
    taps_d = nc.dram_tensor("taps", [P, CG * 9], f32, kind="ExternalInput").ap()
    bias_d = nc.dram_tensor("bias", [P, MG], f32, kind="ExternalInput").ap()
    z_d = nc.dram_tensor("z", [B, C, H, W], f32, kind="ExternalOutput").ap()

    with tile.TileContext(nc) as tc:
        from contextlib import ExitStack
        with ExitStack() as ctx:
            HS_MAX = max(max(r) for r in HS_PE)
            HB_MAX = H - min(min(r) for r in HS_PE)
            HB2_MAX = HP - min(min(r) for r in HS_PE)
            consts = ctx.enter_context(tc.tile_pool(name="consts", bufs=1))
            xpad = ctx.enter_context(tc.tile_pool(name="xpad",
                                                  bufs=BUFS["xpad"]))
            x16bp = ctx.enter_context(tc.tile_pool(name="x16b",
                                                   bufs=BUFS["x16b"]))
            tmpp = ctx.enter_context(tc.tile_pool(name="tmp",
                                                  bufs=BUFS["tmp"]))
            ypep = ctx.enter_context(tc.tile_pool(name="ype",
                                                  bufs=BUFS["ype"]))
            ydvp = ctx.enter_context(tc.tile_pool(name="ydv",
                                                  bufs=BUFS["ydv"]))
            zstp = ctx.enter_context(tc.tile_pool(name="zst",
                                                  bufs=BUFS["zst"]))
            dwps = ctx.enter_context(tc.tile_pool(name="dwps",
                                                  bufs=BUFS["dwps"],
                                                  space="PSUM"))
            pwps = ctx.enter_context(tc.tile_pool(name="pwps",
                                                  bufs=BUFS["pwps"],
                                                  space="PSUM"))

            dwdiag_t = consts.tile([P, CG * 9 * P], f16)
            nc.sync.dma_start(out=dwdiag_t[:], in_=dwdiag_d[:])
            taps_t = consts.tile([P, CG * 9], f32)
            nc.sync.dma_start(out=taps_t[:], in_=taps_d[:])
            # pwT/bias loads are emitted after image 0's input loads (they are
            # not needed until the first pointwise) to unblock PE sooner.
            pwT_t = consts.tile([P, CG * MG * P], f16)
            bias_t = consts.tile([P, MG], f32)
            pw_consts_loaded = [False]

            def load_pw_consts():
                if not pw_consts_loaded[0]:
                    nc.sync.dma_start(out=pwT_t[:], in_=pwT_d[:])
                    nc.sync.dma_start(out=bias_t[:], in_=bias_d[:])
                    pw_consts_loaded[0] = True

            from contextlib import nullcontext
            loop_cm = (tc.For_i(0, hw_loop, 1,
                                hint_engines=(mybir.EngineType.PE,
                                              mybir.EngineType.DVE,
                                              mybir.EngineType.Activation,
                                              mybir.EngineType.SP))
                       if hw_loop else nullcontext())
            with loop_cm:
              for rep in range(reps):

                def emit_dw(i):
                    ype_tiles = []
                    ydv_tiles = []
                    for g in range(CG):
                        HS = HS_PE[i][g]
                        HB = H - HS        # rows on VectorE
                        HB2 = HP - HS      # padded input rows for the DVE band
                        E = HB * W

                        # --- load (contiguous, host-padded, fp16), split so
                        # compute can start on the top rows sooner ---
                        xp = xpad.tile([P, HP, HP], f16)
                        xsrc = x_d[i, g * P:(g + 1) * P, :, :].rearrange(
                            "c a b -> c (a b)")
                        xdst = xp[:, :, :].rearrange("p a b -> p (a b)")
                        HSPLIT = 34 * HP
                        nc.sync.dma_start(out=xdst[:, :HSPLIT],
                                          in_=xsrc[:, :HSPLIT])
                        nc.sync.dma_start(out=xdst[:, HSPLIT:],
                                          in_=xsrc[:, HSPLIT:])

                        # --- depthwise, TensorE rows [0, HS): diag matmuls ---
                        ypet = ypep.tile([P, HS_MAX * W], f16)
                        for h0, nr in _row_chunks(HS):
                            ps = dwps.tile([P, CHUNK], f32)
                            n = nr * W
                            for t, (dh, dw) in enumerate(TAPS):
                                rhs = xp[:, h0 + dh:h0 + dh + nr, dw:dw + W]
                                lhsT = dwdiag_t[:, (g * 9 + t) * P:
                                                (g * 9 + t + 1) * P]
                                nc.tensor.matmul(ps[:, :n], lhsT=lhsT, rhs=rhs,
                                                 start=(t == 0),
                                                 stop=(t == len(TAPS) - 1))
                            nc.scalar.copy(
                                out=ypet[:, h0 * W:h0 * W + n],
                                in_=ps[:, :n],
                            )
                        ype_tiles.append(ypet)

                        # --- shifted-by-one fp16 copy of the DVE band so the
                        # dw=1 taps read 4B-aligned (ScalarE) ---
                        NB = HB2 * HP
                        x16bf = x16bp.tile([P, HB2_MAX * HP], f16)
                        x16b = x16bf[:, :NB].rearrange("p (a b) -> p a b",
                                                       a=HB2)
                        xp_flat = xp[:, :, :].rearrange("p a b -> p (a b)")
                        nc.scalar.copy(
                            out=x16bf[:, 0:NB - 1],
                            in_=xp_flat[:, HS * HP + 1:HS * HP + NB],
                        )

                        # --- depthwise, VectorE rows [HS, 56) ---
                        ydvt = ydvp.tile([P, HB_MAX * W], f16)
                        ydv3 = ydvt[:, :E].rearrange("p (a b) -> p a b", a=HB)

                        def band_ap(dh, dw):
                            if dw == 1:
                                return x16b[:, dh:dh + HB, 0:W]
                            return xp[:, HS + dh:HS + dh + HB, dw:dw + W]

                        sc = lambda t: taps_t[:, g * 9 + t:g * 9 + t + 1]  # noqa: E731
                        nc.vector.tensor_scalar_mul(ydv3[:, :, :],
                                                    band_ap(0, 0), sc(0))
                        for t, (dh, dw) in enumerate(TAPS):
                            if t == 0:
                                continue
                            tmp = tmpp.tile([P, HB_MAX * W], f16)
                            tmp3 = tmp[:, :E].rearrange("p (a b) -> p a b",
                                                        a=HB)
                            nc.vector.tensor_scalar_mul(tmp3[:, :, :],
                                                        band_ap(dh, dw), sc(t))
                            nc.vector.tensor_tensor(ydv3[:, :, :],
                                                    ydv3[:, :, :],
                                                    tmp3[:, :, :], op=Alu.add)
                        ydv_tiles.append(ydvt)
                    return ype_tiles, ydv_tiles

                def emit_pw(i, ype_tiles, ydv_tiles):
                    load_pw_consts()
                    for mg in range(MG):
                        for r0, nr in _row_chunks(H):
                            n = nr * W
                            ps = pwps.tile([P, CHUNK], f32)
                            mms = []
                            for kg in range(CG):
                                hs = HS_PE[i][kg]
                                pieces = []
                                if r0 < hs:
                                    pr = min(nr, hs - r0)
                                    pieces.append((r0, pr, ype_tiles[kg],
                                                   r0 * W))
                                    if pr < nr:
                                        pieces.append((r0 + pr, nr - pr,
                                                       ydv_tiles[kg], 0))
                                else:
                                    pieces.append((r0, nr, ydv_tiles[kg],
                                                   (r0 - hs) * W))
                                for pr0, pnr, srct, soff in pieces:
                                    mms.append((kg, pr0, pnr, srct, soff))
                            for j, (kg, pr0, pnr, srct, soff) in enumerate(mms):
                                rhs = srct[:, soff:soff + pnr * W]
                                out_ap = ps[:, (pr0 - r0) * W:
                                            (pr0 - r0) * W + pnr * W]
                                nc.tensor.matmul(
                                    out_ap,
                                    lhsT=pwT_t[:, (kg * MG + mg) * P:
                                               (kg * MG + mg + 1) * P],
                                    rhs=rhs,
                                    start=(j == 0),
                                    stop=(j == len(mms) - 1),
                                    skip_group_check=True,
                                )
                            zt = zstp.tile([P, CHUNK], f32)
                            nc.scalar.activation(
                                out=zt[:, :n],
                                in_=ps[:, :n],
                                func=Act.Identity,
                                bias=bias_t[:, mg:mg + 1],
                                scale=1.0,
                            )
                            nc.sync.dma_start(
                                out=z_d[i, mg * P:(mg + 1) * P,
                                        r0:r0 + nr, :],
                                in_=zt[:, :n].rearrange("p (a b) -> p a b",
                                                        a=nr),
                            )

                for i in range(B):
                    ype_tiles, ydv_tiles = emit_dw(i)
                    emit_pw(i, ype_tiles, ydv_tiles)

    nc.compile()
    return nc


def _host_consts(dw_w: np.ndarray, pw_w: np.ndarray, pw_b: np.ndarray):
    dw_q = _fake_quant(dw_w)                      # [384, 1, 3, 3]
    pw_q = _fake_quant(pw_w)                      # [384, 384, 1, 1]

    # taps [128, CG*9]: [c, g*9 + t] = dw_q[g*128 + c, 0, dh, dw]
    taps = (dw_q[:, 0].reshape(C, 9).reshape(CG, P, 9)
            .transpose(1, 0, 2).reshape(P, CG * 9).astype(np.float32))
    taps = np.ascontiguousarray(taps)

    # dwdiag [128, CG*9*128] fp16: block (g*9+t) = diag of that tap's weights
    eye = np.eye(P, dtype=np.float16)
    blocks = []
    for g in range(CG):
        for t in range(9):
            d = taps[:, g * 9 + t].astype(np.float16)
            blocks.append(eye * d[:, None])
    dwdiag = np.ascontiguousarray(np.concatenate(blocks, axis=1))

    # pwT [128, CG*MG*128] fp16: block (kg*MG+mg)[k, m] = pw_q[mg*128+m, kg*128+k]
    pw2 = pw_q[:, :, 0, 0]
    blocks = []
    for kg in range(CG):
        for mg in range(MG):
            blocks.append(np.ascontiguousarray(
                pw2[mg * P:(mg + 1) * P, kg * P:(kg + 1) * P].T.astype(np.float16)))
    pwT = np.ascontiguousarray(np.concatenate(blocks, axis=1))

    bias = np.ascontiguousarray(
        pw_b.astype(np.float32).reshape(MG, P).T.astype(np.float32))
    return dwdiag, pwT, taps, bias


def _prepare_in_maps(x, dw_w, pw_w, pw_b):
    dwdiag, pwT, taps, bias = _host_consts(dw_w, pw_w, pw_b)

    x = np.asarray(x, dtype=np.float32)
    xp = np.zeros((B_TOTAL, C, HP, HP), dtype=np.float16)
    xp[:, :, 1:H + 1, 1:W + 1] = x.astype(np.float16)
    shards = xp.reshape(N_CORES, B, C, HP, HP)
    return [
        {"x": np.ascontiguousarray(shards[c]), "dwdiag": dwdiag, "pwT": pwT,
         "taps": taps, "bias": bias}
        for c in range(N_CORES)
    ]


_NC_CACHE = None


def kernel(x: np.ndarray, dw_w: np.ndarray, pw_w: np.ndarray,
           pw_b: np.ndarray) -> np.ndarray:
    from concourse.bass_utils import run_bass_kernel_spmd

    global _NC_CACHE
    if _NC_CACHE is None:
        _NC_CACHE = _build_nc()
    nc = _NC_CACHE

    in_maps = _prepare_in_maps(x, dw_w, pw_w, pw_b)
    res = run_bass_kernel_spmd(nc, in_maps, list(range(N_CORES)))
    z = np.concatenate([res.results[c]["z"] for c in range(N_CORES)], axis=0)
    return z.astype(np.float32)



# revision 2
# speedup vs baseline: 38.0704x; 38.0704x over previous
"""Depthwise 3x3 + pointwise 1x1 conv (both 4-bit fake-quant weights) on 8 trn2 cores.

Strategy: data-parallel over batch (32 -> 8 cores x 4 images).
Output computed/stored as fp16 (upcast to fp32 on host): halves the z store
traffic. z accumulated per (image, out-group) into one [128, 3136] tile and
stored with a single DMA (12 stores instead of 84; keeps the SP queue free
for input loads). Depthwise PSUM chunks are 9 rows (504 of 512 bank slots).
Host pre-pads x to 58x58 (zero border) and converts it to fp16, so every DMA
load is one fully contiguous ~6.7KB-per-partition transfer and no on-device
dtype conversion of the input is needed.
Per core: channels-on-partitions layout (384 ch = 3 groups of 128).
 - depthwise: 9 taps, split by output rows between TensorE and VectorE
   (per-image/per-group row split in HS_PE, tuned for engine balance):
   * TensorE rows [0, HS): one diagonal-weight fp16 matmul per tap,
     accumulated in a PSUM bank over the 9 taps (8-row/448-col chunks),
     evacuated to fp16 on ScalarE.
   * VectorE rows [HS, 56): per-tap tensor_scalar products (4x DVE mode,
     per-partition tap scalars) + tensor_tensor adds (2x mode). ScalarE
     makes a shifted-by-one fp16 copy of the band so the center-column
     (dw=1) taps also read 4-byte-aligned.
 - pointwise: dense fp16 matmuls on TensorE (K=384 = 3x128 accumulated in
   fp32 PSUM), bias added on ScalarE (Identity activation + per-partition
   bias AP) during PSUM evacuation, stored as fp32.
Weight fake-quant is replicated on host in fp32 numpy (exact match to the
reference semantics: forward value of the STE round).
"""

import numpy as np

# Problem shape (hardcoded per contract).
B_TOTAL, C, H, W = 32, 384, 56, 56
N_CORES = 8
B = B_TOTAL // N_CORES          # images per core
HP = H + 2                      # padded spatial
NPAD = HP * HP
CG = C // 128                   # channel groups (contraction groups)
MG = C // 128                   # output-channel groups
P = 128

# Depthwise rows computed on TensorE, per (image, channel-group); the rest of
# the 56 rows go to VectorE. Image 0 leans on VectorE (PE starts before DVE
# has data), image 3 leans on TensorE (so the kernel tail isn't gated on the
# last VectorE band).
HS_PE = [
    [27, 27, 27],
    [36, 27, 27],
    [36, 27, 27],
    [36, 36, 36],
]
BUFS = {"xpad": 6, "x16b": 4, "tmp": 4, "ype": 9, "ydv": 9, "zst": 4,
        "dwps": 4, "pwps": 4}
CHUNK_ROWS = 9                  # rows per PSUM chunk (fp32 bank: 512 elems)
CHUNK = CHUNK_ROWS * W          # 504 columns


def _row_chunks(total, step=CHUNK_ROWS):
    out, r = [], 0
    while r < total:
        n = min(step, total - r)
        out.append((r, n))
        r += n
    return out

TAPS = [(dh, dw) for dh in range(3) for dw in range(3)]

WEIGHT_BITS = 4
SCALE_MIN = np.float32(2e-16)


def _fake_quant(w: np.ndarray, bits: int = WEIGHT_BITS) -> np.ndarray:
    """Forward value of brevitas-style per-channel symmetric narrow int quant."""
    w = w.astype(np.float32)
    qmax = np.float32(2 ** (bits - 1) - 1)
    absmax = np.max(np.abs(w.reshape(w.shape[0], -1)), axis=1)
    scale = np.maximum(absmax / qmax, SCALE_MIN).astype(np.float32)
    scale = scale.reshape((-1,) + (1,) * (w.ndim - 1))
    q = np.clip(np.round(w / scale), -qmax, qmax).astype(np.float32) * scale
    return q.astype(np.float32)


def _build_nc(reps: int = 1, hw_loop: int = 0):
    import concourse.bass as bass  # noqa: F401
    import concourse.tile as tile
    from concourse import bacc, mybir

    dt = mybir.dt
    f32, f16, f32r = dt.float32, dt.float16, dt.float32r
    Alu = mybir.AluOpType
    Act = mybir.ActivationFunctionType

    nc = bacc.Bacc("TRN2", target_bir_lowering=False, debug=False,
                   num_devices=N_CORES)

    # x arrives host-padded and host-converted: [B, C, 58, 58] fp16,
    # zero borders.
    x_d = nc.dram_tensor("x", [B, C, HP, HP], f16, kind="ExternalInput").ap()
    dwdiag_d = nc.dram_tensor("dwdiag", [P, CG * 9 * P], f16,
                              kind="ExternalInput").ap()
    pwT_d = nc.dram_tensor("pwT", [P, CG * MG * P], f16,
                           kind="ExternalInput").ap()
    taps_d = nc.dram_tensor("taps", [P, CG * 9], f32, kind="ExternalInput").ap()
    bias_d = nc.dram_tensor("bias", [P, MG], f32, kind="ExternalInput").ap()
    z_d = nc.dram_tensor("z", [B, C, H, W], f16, kind="ExternalOutput").ap()

    with tile.TileContext(nc) as tc:
        from contextlib import ExitStack
        with ExitStack() as ctx:
            HS_MAX = max(max(r) for r in HS_PE)
            HB_MAX = H - min(min(r) for r in HS_PE)
            HB2_MAX = HP - min(min(r) for r in HS_PE)
            consts = ctx.enter_context(tc.tile_pool(name="consts", bufs=1))
            xpad = ctx.enter_context(tc.tile_pool(name="xpad",
                                                  bufs=BUFS["xpad"]))
            x16bp = ctx.enter_context(tc.tile_pool(name="x16b",
                                                   bufs=BUFS["x16b"]))
            tmpp = ctx.enter_context(tc.tile_pool(name="tmp",
                                                  bufs=BUFS["tmp"]))
            ypep = ctx.enter_context(tc.tile_pool(name="ype",
                                                  bufs=BUFS["ype"]))
            ydvp = ctx.enter_context(tc.tile_pool(name="ydv",
                                                  bufs=BUFS["ydv"]))
            zstp = ctx.enter_context(tc.tile_pool(name="zst",
                                                  bufs=BUFS["zst"]))
            dwps = ctx.enter_context(tc.tile_pool(name="dwps",
                                                  bufs=BUFS["dwps"],
                                                  space="PSUM"))
            pwps = ctx.enter_context(tc.tile_pool(name="pwps",
                                                  bufs=BUFS["pwps"],
                                                  space="PSUM"))

            dwdiag_t = consts.tile([P, CG * 9 * P], f16)
            taps_t = consts.tile([P, CG * 9], f32)
            nc.scalar.dma_start(out=dwdiag_t[:, 0:9 * P],
                                in_=dwdiag_d[:, 0:9 * P])
            nc.scalar.dma_start(out=taps_t[:], in_=taps_d[:])
            for _g in range(1, CG):
                nc.scalar.dma_start(
                    out=dwdiag_t[:, _g * 9 * P:(_g + 1) * 9 * P],
                    in_=dwdiag_d[:, _g * 9 * P:(_g + 1) * 9 * P])
            # pwT/bias loads are emitted after image 0's input loads (they are
            # not needed until the first pointwise) to unblock PE sooner.
            pwT_t = consts.tile([P, CG * MG * P], f16)
            bias_t = consts.tile([P, MG], f32)
            pw_consts_loaded = [False]

            def load_pw_consts():
                if not pw_consts_loaded[0]:
                    nc.sync.dma_start(out=pwT_t[:], in_=pwT_d[:])
                    nc.sync.dma_start(out=bias_t[:], in_=bias_d[:])
                    pw_consts_loaded[0] = True

            from contextlib import nullcontext
            loop_cm = (tc.For_i(0, hw_loop, 1,
                                hint_engines=(mybir.EngineType.PE,
                                              mybir.EngineType.DVE,
                                              mybir.EngineType.Activation,
                                              mybir.EngineType.SP))
                       if hw_loop else nullcontext())
            with loop_cm:
              for rep in range(reps):

                def emit_dw(i):
                    ype_tiles = []
                    ydv_tiles = []
                    for g in range(CG):
                        HS = HS_PE[i][g]
                        HB = H - HS        # rows on VectorE
                        HB2 = HP - HS      # padded input rows for the DVE band
                        E = HB * W

                        # --- load (contiguous, host-padded, fp16), split so
                        # compute can start on the top rows sooner ---
                        xp = xpad.tile([P, HP, HP], f16)
                        xsrc = x_d[i, g * P:(g + 1) * P, :, :].rearrange(
                            "c a b -> c (a b)")
                        xdst = xp[:, :, :].rearrange("p a b -> p (a b)")
                        if i == 0 and g == 0:
                            HSPLIT = 20 * HP
                            nc.sync.dma_start(out=xdst[:, :HSPLIT],
                                              in_=xsrc[:, :HSPLIT])
                            nc.sync.dma_start(out=xdst[:, HSPLIT:],
                                              in_=xsrc[:, HSPLIT:])
                        else:
                            nc.sync.dma_start(out=xdst[:], in_=xsrc[:])

                        # --- depthwise, TensorE rows [0, HS): diag matmuls ---
                        ypet = ypep.tile([P, HS_MAX * W], f16)
                        for h0, nr in _row_chunks(HS):
                            ps = dwps.tile([P, CHUNK], f32)
                            n = nr * W
                            for t, (dh, dw) in enumerate(TAPS):
                                rhs = xp[:, h0 + dh:h0 + dh + nr, dw:dw + W]
                                lhsT = dwdiag_t[:, (g * 9 + t) * P:
                                                (g * 9 + t + 1) * P]
                                nc.tensor.matmul(ps[:, :n], lhsT=lhsT, rhs=rhs,
                                                 start=(t == 0),
                                                 stop=(t == len(TAPS) - 1))
                            nc.scalar.copy(
                                out=ypet[:, h0 * W:h0 * W + n],
                                in_=ps[:, :n],
                            )
                        ype_tiles.append(ypet)

                        # --- shifted-by-one fp16 copy of the DVE band so the
                        # dw=1 taps read 4B-aligned (ScalarE) ---
                        NB = HB2 * HP
                        x16bf = x16bp.tile([P, HB2_MAX * HP], f16)
                        x16b = x16bf[:, :NB].rearrange("p (a b) -> p a b",
                                                       a=HB2)
                        xp_flat = xp[:, :, :].rearrange("p a b -> p (a b)")
                        nc.scalar.copy(
                            out=x16bf[:, 0:NB - 1],
                            in_=xp_flat[:, HS * HP + 1:HS * HP + NB],
                        )

                        # --- depthwise, VectorE rows [HS, 56) ---
                        ydvt = ydvp.tile([P, HB_MAX * W], f16)
                        ydv3 = ydvt[:, :E].rearrange("p (a b) -> p a b", a=HB)

                        def band_ap(dh, dw):
                            if dw == 1:
                                return x16b[:, dh:dh + HB, 0:W]
                            return xp[:, HS + dh:HS + dh + HB, dw:dw + W]

                        sc = lambda t: taps_t[:, g * 9 + t:g * 9 + t + 1]  # noqa: E731
                        nc.vector.tensor_scalar_mul(ydv3[:, :, :],
                                                    band_ap(0, 0), sc(0))
                        for t, (dh, dw) in enumerate(TAPS):
                            if t == 0:
                                continue
                            tmp = tmpp.tile([P, HB_MAX * W], f16)
                            tmp3 = tmp[:, :E].rearrange("p (a b) -> p a b",
                                                        a=HB)
                            nc.vector.tensor_scalar_mul(tmp3[:, :, :],
                                                        band_ap(dh, dw), sc(t))
                            nc.vector.tensor_tensor(ydv3[:, :, :],
                                                    ydv3[:, :, :],
                                                    tmp3[:, :, :], op=Alu.add)
                        ydv_tiles.append(ydvt)
                    return ype_tiles, ydv_tiles

                def emit_pw(i, ype_tiles, ydv_tiles):
                    load_pw_consts()
                    for mg in range(MG):
                        zt = zstp.tile([P, H * W], f16)
                        for r0, nr in _row_chunks(H):
                            n = nr * W
                            ps = pwps.tile([P, CHUNK], f32)
                            mms = []
                            for kg in range(CG):
                                hs = HS_PE[i][kg]
                                pieces = []
                                if r0 < hs:
                                    pr = min(nr, hs - r0)
                                    pieces.append((r0, pr, ype_tiles[kg],
                                                   r0 * W))
                                    if pr < nr:
                                        pieces.append((r0 + pr, nr - pr,
                                                       ydv_tiles[kg], 0))
                                else:
                                    pieces.append((r0, nr, ydv_tiles[kg],
                                                   (r0 - hs) * W))
                                for pr0, pnr, srct, soff in pieces:
                                    mms.append((kg, pr0, pnr, srct, soff))
                            for j, (kg, pr0, pnr, srct, soff) in enumerate(mms):
                                rhs = srct[:, soff:soff + pnr * W]
                                out_ap = ps[:, (pr0 - r0) * W:
                                            (pr0 - r0) * W + pnr * W]
                                nc.tensor.matmul(
                                    out_ap,
                                    lhsT=pwT_t[:, (kg * MG + mg) * P:
                                               (kg * MG + mg + 1) * P],
                                    rhs=rhs,
                                    start=(j == 0),
                                    stop=(j == len(mms) - 1),
                                    skip_group_check=True,
                                )
                            if i == B - 1 and mg == MG - 1 and (r0 // 9) % 2:
                                nc.vector.tensor_scalar_add(
                                    zt[:, r0 * W:r0 * W + n], ps[:, :n],
                                    bias_t[:, mg:mg + 1])
                            else:
                                nc.scalar.activation(
                                    out=zt[:, r0 * W:r0 * W + n],
                                    in_=ps[:, :n],
                                    func=Act.Identity,
                                    bias=bias_t[:, mg:mg + 1],
                                    scale=1.0,
                                )
                        if i == B - 1:
                            HSP = 28 * W
                            zdf = z_d[i, mg * P:(mg + 1) * P, :, :].rearrange(
                                "c a b -> c (a b)")
                            nc.sync.dma_start(out=zdf[:, :HSP],
                                              in_=zt[:, :HSP])
                            nc.sync.dma_start(out=zdf[:, HSP:],
                                              in_=zt[:, HSP:])
                        else:
                            nc.sync.dma_start(
                                out=z_d[i, mg * P:(mg + 1) * P, :, :]
                                .rearrange("c a b -> c (a b)"),
                                in_=zt[:, :],
                            )

                for i in range(B):
                    ype_tiles, ydv_tiles = emit_dw(i)
                    emit_pw(i, ype_tiles, ydv_tiles)

    nc.compile()
    return nc


def _host_consts(dw_w: np.ndarray, pw_w: np.ndarray, pw_b: np.ndarray):
    dw_q = _fake_quant(dw_w)                      # [384, 1, 3, 3]
    pw_q = _fake_quant(pw_w)                      # [384, 384, 1, 1]

    # taps [128, CG*9]: [c, g*9 + t] = dw_q[g*128 + c, 0, dh, dw]
    taps = (dw_q[:, 0].reshape(C, 9).reshape(CG, P, 9)
            .transpose(1, 0, 2).reshape(P, CG * 9).astype(np.float32))
    taps = np.ascontiguousarray(taps)

    # dwdiag [128, CG*9*128] fp16: block (g*9+t) = diag of that tap's weights
    eye = np.eye(P, dtype=np.float16)
    blocks = []
    for g in range(CG):
        for t in range(9):
            d = taps[:, g * 9 + t].astype(np.float16)
            blocks.append(eye * d[:, None])
    dwdiag = np.ascontiguousarray(np.concatenate(blocks, axis=1))

    # pwT [128, CG*MG*128] fp16: block (kg*MG+mg)[k, m] = pw_q[mg*128+m, kg*128+k]
    pw2 = pw_q[:, :, 0, 0]
    blocks = []
    for kg in range(CG):
        for mg in range(MG):
            blocks.append(np.ascontiguousarray(
                pw2[mg * P:(mg + 1) * P, kg * P:(kg + 1) * P].T.astype(np.float16)))
    pwT = np.ascontiguousarray(np.concatenate(blocks, axis=1))

    bias = np.ascontiguousarray(
        pw_b.astype(np.float32).reshape(MG, P).T.astype(np.float32))
    return dwdiag, pwT, taps, bias


def _prepare_in_maps(x, dw_w, pw_w, pw_b):
    dwdiag, pwT, taps, bias = _host_consts(dw_w, pw_w, pw_b)

    x = np.asarray(x, dtype=np.float32)
    xp = np.zeros((B_TOTAL, C, HP, HP), dtype=np.float16)
    xp[:, :, 1:H + 1, 1:W + 1] = x.astype(np.float16)
    shards = xp.reshape(N_CORES, B, C, HP, HP)
    return [
        {"x": np.ascontiguousarray(shards[c]), "dwdiag": dwdiag, "pwT": pwT,
         "taps": taps, "bias": bias}
        for c in range(N_CORES)
    ]


_NC_CACHE = None


def kernel(x: np.ndarray, dw_w: np.ndarray, pw_w: np.ndarray,
           pw_b: np.ndarray) -> np.ndarray:
    from concourse.bass_utils import run_bass_kernel_spmd

    global _NC_CACHE
    if _NC_CACHE is None:
        _NC_CACHE = _build_nc()
    nc = _NC_CACHE

    in_maps = _prepare_in_maps(x, dw_w, pw_w, pw_b)
    res = run_bass_kernel_spmd(nc, in_maps, list(range(N_CORES)))
    z = np.concatenate([res.results[c]["z"] for c in range(N_CORES)], axis=0)
    return z.astype(np.float32)

